# revision 14
# baseline (speedup 1.0000x reference)
"""Trainium2 Bass kernel for the CAN capsule-routing module (nn_CAN_12446815223813).

Self-contained: hardcodes problem shapes, shards batch B=8 across 8 NeuronCores
(pure data parallel, one batch element per core), runs a Tile/Bass kernel via
run_bass_kernel_spmd, and gathers the full [8, 32, 8, 23] output.

Algorithm (mathematically identical to the reference, factored):
  * The NI axis of `hat` is a pure broadcast and the joint softmax over
    (NI,NC,NP) just scales the denominator by NI=8; outputs are identical
    across NI, so routing runs once per (NC,NP) and the result is broadcast.
  * ppart (x[...,0]) never reaches the output -> dropped.
  * hat[B,M,K,DC] is never materialized. With xfeat[j,f] = [x_geo(6), 1,
    x_attr(16)] (j = IC*II positions, f = 23 feats) and W1f[cp,7,6],
    W2f[cp,16,16] (cp = NC*NP):
      per routing iter (b-logits kept transposed as b[j, cp], accumulated
      directly in PSUM by the agree matmuls):
        e = exp(b);  r = 1/(8 * rowsum(e))          (|b| <= ~12: no max-sub)
        CF[cp,:] = e.T @ (xfeat * r/8)              (bf16 matmul, f32 psum)
        Sga[cp,:] = [cg @ W1f[cp], ca @ W2f[cp], sumc]   (DVE mul+reduce)
        O23[c,:]  = sum_p Sga[(c,p),:]              (ones-selector matmul)
        outputs   = squash(O23)    (rsqrt via exp(-0.5*ln(.)): one ACT table)
        V[cp,:]   = [W1f[cp] @ og(c), 0.01 * W2f[cp] @ oa(c)]
        b        += xfeat @ V.T    (bf16 matmul accumulating into PSUM)

Hardware notes (found by bisection on trn2):
  * ScalarE ACTIVATE reading PSUM with partial-partition tiles crashes the
    exec unit -> PSUM reads happen on VectorE/TensorE, except the full
    128-partition exp reads of the b PSUM tiles.
  * tensor_tensor_reduce crashes the exec unit -> mul + reduce pair instead.
  * All constant inputs arrive as one [128, F] blob -> single input DMA.
"""

import numpy as np
import ml_dtypes

import concourse.bass as bass
import concourse.bacc as bacc
import concourse.tile as tile
from concourse import bass_utils, mybir

F32 = mybir.dt.float32
BF16 = mybir.dt.bfloat16
AF = mybir.ActivationFunctionType
ALU = mybir.AluOpType
AX = mybir.AxisListType

B, IC, II, ID = 8, 16, 16, 23
NC, NP, DA = 32, 8, 16
NI = 8
DIM_GEOM = 6
DC = 1 + DIM_GEOM + DA  # 23
J = IC * II             # 256 positions
CP = NC * NP            # 256 capsule-part pairs
EPS = 1e-7
ROUTINGS = 3
N_CORES = 8

# blob column layout (f32, [128, FBLOB])
_C_XF = 0                    # xf chunks: [0:23], [23:46]
_C_W1GD = 46                 # w1 [p,g,d]: 46:88, 88:130
_C_W1DG = 130                # w1 [p,d,g]: 130:172, 172:214
_C_W2AD = 214                # w2 [p,a,d]: 214:470, 470:726
_C_W2DA = 726                # w2 [p,d,a]: 726:982, 982:1238
_C_SELT = 1238               # selT: 1238:1270, 1270:1302
_C_IDENT = 1302              # identity: 1302:1430
_C_BSEL = 1430               # bsel (rows 0:32): 1430:1686
FBLOB = 1686

_CACHE = {}

# accumulate b-logits in PSUM (HW-verified; CoreSim's accumulation-group
# model rejects mid-group PSUM reads, so simtest sets this False).
# NOTE: HW-verified that ScalarE exp reading PSUM crashes the exec unit
# (NRT_EXEC_UNIT_UNRECOVERABLE 101), so this stays False.
PSUM_B = False


def _build_nc(psum_b=None):
    psum_b = PSUM_B if psum_b is None else psum_b
    nc = bacc.Bacc("TRN2", target_bir_lowering=False, debug=False,
                   num_devices=N_CORES)

    blob_d = nc.dram_tensor("blob", [128, FBLOB], F32, kind="ExternalInput")
    xfT_d = nc.dram_tensor("xfT", [DC, J], BF16, kind="ExternalInput")
    out_d = nc.dram_tensor("out", [NC, NI * DC], F32, kind="ExternalOutput")

    with tile.TileContext(nc) as tc:
        with (
            tc.tile_pool(name="sb", bufs=1) as sb,
            tc.tile_pool(name="pmm", bufs=2, space="PSUM") as pmm,   # CF/OGA [128,23]
            tc.tile_pool(name="po", bufs=1, space="PSUM") as po,     # O23 [32,23]
            tc.tile_pool(name="pvt", bufs=2, space="PSUM") as pvt,   # VT [23,128]
            tc.tile_pool(name="pb", bufs=1, space="PSUM") as pb,     # b logits [128,256]
        ):
            # ---- persistent SBUF tiles ----
            mega = sb.tile([128, FBLOB], F32, name="mega", tag="mega")
            xfT = sb.tile([DC, J], BF16, name="xfT", tag="xfT")

            e = [sb.tile([128, CP], BF16, name=f"e{k}", tag=f"e{k}") for k in range(2)]
            xfr = [sb.tile([128, DC], BF16, name=f"xfr{k}", tag=f"xfr{k}") for k in range(2)]
            rsum = [sb.tile([128, 1], F32, name=f"rsum{k}", tag=f"rsum{k}") for k in range(2)]
            rrec = [sb.tile([128, 1], F32, name=f"rrec{k}", tag=f"rrec{k}") for k in range(2)]
            tmpg = [sb.tile([128, 42], F32, name=f"tmpg{k}", tag=f"tmpg{k}") for k in range(2)]
            tmpa = [sb.tile([128, 256], F32, name=f"tmpa{k}", tag=f"tmpa{k}") for k in range(2)]
            sga = [sb.tile([128, DC], F32, name=f"sga{k}", tag=f"sga{k}") for k in range(2)]
            vt_s = [sb.tile([128, DC], F32, name=f"vt{k}", tag=f"vt{k}") for k in range(2)]
            vT = sb.tile([DC, CP], BF16, name="vT", tag="vT")
            o23 = sb.tile([NC, DC], F32, name="o23", tag="o23")
            oout = sb.tile([NC, DC], F32, name="oout", tag="oout")
            orep = sb.tile([NC, NI * DC], F32, name="orep", tag="orep")
            sc_r = sb.tile([NC, 1], F32, name="sc_r", tag="sc_r")
            s2 = sb.tile([NC, 1], F32, name="s2", tag="s2")
            sq = sb.tile([NC, 16], F32, name="sq", tag="sq")
            t1 = sb.tile([NC, 1], F32, name="t1", tag="t1")
            r1 = sb.tile([NC, 1], F32, name="r1", tag="r1")
            lnt = sb.tile([NC, 1], F32, name="lnt", tag="lnt")
            r2 = sb.tile([NC, 1], F32, name="r2", tag="r2")
            epsb = sb.tile([NC, 1], F32, name="epsb", tag="epsb")
            zerob = sb.tile([128, 1], F32, name="zerob", tag="zerob")

            # b logits live in PSUM; agree matmuls accumulate into them
            bps = [pb.tile([128, CP], F32, name=f"bps{k}", tag=f"bps{k}")
                   for k in range(2)]
            if not psum_b:
                bT = [sb.tile([128, CP], F32, name=f"bT{k}", tag=f"bT{k}")
                      for k in range(2)]

            # ---- load all inputs in two DMAs ----
            nc.sync.dma_start(mega[:], blob_d[:, :])
            nc.sync.dma_start(xfT[:], xfT_d[:, :])

            # views into the blob
            def col(c0, w):
                return mega[:, c0:c0 + w]
            xf = [col(_C_XF + 23 * k, 23) for k in range(2)]
            w1_gd = [col(_C_W1GD + 42 * k, 42).rearrange("p (g d) -> p g d", d=6)
                     for k in range(2)]
            w1_dg = [col(_C_W1DG + 42 * k, 42).rearrange("p (d g) -> p d g", g=7)
                     for k in range(2)]
            w2_ad = [col(_C_W2AD + 256 * k, 256).rearrange("p (a d) -> p a d", d=16)
                     for k in range(2)]
            w2_da = [col(_C_W2DA + 256 * k, 256).rearrange("p (d a) -> p d a", a=16)
                     for k in range(2)]
            selT = [col(_C_SELT + 32 * k, 32) for k in range(2)]
            ident = col(_C_IDENT, 128)
            bsel = mega[0:NC, _C_BSEL:_C_BSEL + 256]

            nc.gpsimd.memset(epsb[:], EPS)
            nc.gpsimd.memset(zerob[:], 0.0)
            # t=0: uniform routing weights (b=0) -> e=1, r=1/(8*256)
            for k in range(2):
                nc.gpsimd.memset(e[k][:], 1.0)

            for t in range(ROUTINGS):
                # -- normalized position weights --
                if t == 0:
                    for k in range(2):
                        nc.vector.tensor_scalar_mul(xfr[k][:], xf[k],
                                                    1.0 / (NI * CP))
                else:
                    for k in range(2):
                        # e = exp(b) straight from PSUM; rowsum via accumulator
                        nc.scalar.activation(e[k][:],
                                             bps[k][:] if psum_b else bT[k][:],
                                             AF.Exp, bias=zerob[:], scale=1.0,
                                             accum_out=rsum[k][:])
                        nc.vector.reciprocal(rrec[k][:], rsum[k][:])
                        # xfr = xf * rrec / NI
                        nc.vector.tensor_scalar(xfr[k][:], xf[k], rrec[k][:],
                                                1.0 / NI, op0=ALU.mult,
                                                op1=ALU.mult)

                # -- CF[cp, f] = sum_j c[cp,j] * xfeat[j,f]  (e.T @ xfr) --
                cf = []
                for m in range(2):
                    cfm = pmm.tile([128, DC], F32, name="pmm", tag="pmm")
                    for k in range(2):
                        nc.tensor.matmul(cfm[:], e[k][:, m * 128:(m + 1) * 128],
                                         xfr[k][:], start=(k == 0), stop=(k == 1))
                    cf.append(cfm)

                # -- Sga[cp] = [cg @ W1f, ca @ W2f, sumc] --
                for m in range(2):
                    cg_b = cf[m][:, 0:7].unsqueeze(1).broadcast_to([128, 6, 7])
                    nc.vector.scalar_tensor_tensor(
                        tmpg[m][:].rearrange("p (d g) -> p d g", g=7),
                        cg_b, 1.0, w1_dg[m], op0=ALU.mult, op1=ALU.mult)
                    nc.vector.tensor_reduce(
                        sga[m][:, 0:6], tmpg[m][:].rearrange("p (d g) -> p d g", g=7),
                        axis=AX.X, op=ALU.add)
                    ca_b = cf[m][:, 7:23].unsqueeze(1).broadcast_to([128, 16, 16])
                    nc.vector.scalar_tensor_tensor(
                        tmpa[m][:].rearrange("p (d a) -> p d a", a=16),
                        ca_b, 1.0, w2_da[m], op0=ALU.mult, op1=ALU.mult)
                    nc.vector.tensor_reduce(
                        sga[m][:, 6:22], tmpa[m][:].rearrange("p (d a) -> p d a", a=16),
                        axis=AX.X, op=ALU.add)
                    nc.vector.tensor_copy(sga[m][:, 22:23], cf[m][:, 6:7])

                # -- O23[c] = sum_p Sga[(c,p)] --
                o23p = po.tile([NC, DC], F32, name="po", tag="po")
                for m in range(2):
                    nc.tensor.matmul(o23p[:], selT[m], sga[m][:],
                                     start=(m == 0), stop=(m == 1))
                nc.vector.tensor_copy(o23[:], o23p[:])

                # -- squash -> outputs [32, 23] --
                nc.vector.reciprocal(sc_r[:], o23[:, 22:23])
                nc.vector.tensor_scalar_mul(oout[:, 1:7], o23[:, 0:6], sc_r[:])
                nc.vector.tensor_tensor(sq[:], o23[:, 6:22], o23[:, 6:22],
                                        op=ALU.mult)
                nc.vector.reduce_sum(s2[:], sq[:], axis=AX.X)
                nc.vector.tensor_scalar_add(t1[:], s2[:], 1.0)
                nc.vector.reciprocal(r1[:], t1[:])
                # rsqrt(s2+eps) = exp(-0.5*ln(s2+eps)): stays in one ACT table set
                nc.scalar.activation(lnt[:], s2[:], AF.Ln, bias=epsb[:], scale=1.0)
                nc.scalar.activation(r2[:], lnt[:], AF.Exp, bias=zerob[0:NC, :],
                                     scale=-0.5)
                # scale = s2 * r1 * r2, written straight into outputs col 0
                nc.vector.scalar_tensor_tensor(oout[:, 0:1], r1[:], s2[:], r2[:],
                                               op0=ALU.mult, op1=ALU.mult)
                nc.vector.tensor_scalar_mul(oout[:, 7:23], o23[:, 6:22],
                                            oout[:, 0:1])

                if t < ROUTINGS - 1:
                    # -- V[cp] = [W1f @ og, 0.01 * W2f @ oa] --
                    for m in range(2):
                        ogam = pmm.tile([128, DC], F32, name="pmm", tag="pmm")
                        nc.tensor.matmul(ogam[:], bsel[:, m * 128:(m + 1) * 128],
                                         oout[:], start=True, stop=True)
                        og_b = ogam[:, 1:7].unsqueeze(1).broadcast_to([128, 7, 6])
                        nc.vector.scalar_tensor_tensor(
                            tmpg[m][:].rearrange("p (g d) -> p g d", d=6),
                            og_b, 1.0, w1_gd[m], op0=ALU.mult, op1=ALU.mult)
                        nc.vector.tensor_reduce(
                            vt_s[m][:, 0:7],
                            tmpg[m][:].rearrange("p (g d) -> p g d", d=6),
                            axis=AX.X, op=ALU.add)
                        oa_b = ogam[:, 7:23].unsqueeze(1).broadcast_to([128, 16, 16])
                        nc.vector.scalar_tensor_tensor(
                            tmpa[m][:].rearrange("p (a d) -> p a d", d=16),
                            oa_b, 0.01, w2_ad[m], op0=ALU.mult, op1=ALU.mult)
                        nc.vector.tensor_reduce(
                            vt_s[m][:, 7:23],
                            tmpa[m][:].rearrange("p (a d) -> p a d", d=16),
                            axis=AX.X, op=ALU.add)
                        # transpose V chunk -> vT[:, chunk] (bf16 for agree matmul)
                        vtp = pvt.tile([DC, 128], F32, name="pvt", tag="pvt")
                        nc.tensor.transpose(vtp[:], vt_s[m][:], ident)
                        nc.vector.tensor_copy(vT[:, m * 128:(m + 1) * 128], vtp[:])

                    # -- agree[j, cp] = xfeat @ V.T, accumulated into b PSUM --
                    for k in range(2):
                        if psum_b:
                            nc.tensor.matmul(bps[k][:],
                                             xfT[:, k * 128:(k + 1) * 128],
                                             vT[:], start=(t == 0),
                                             stop=(t == 1),
                                             skip_group_check=True)
                        else:
                            nc.tensor.matmul(bps[k][:],
                                             xfT[:, k * 128:(k + 1) * 128],
                                             vT[:], start=True, stop=True)
                            if t == 0:
                                nc.vector.tensor_copy(bT[k][:], bps[k][:])
                            else:
                                nc.vector.tensor_tensor(bT[k][:], bT[k][:],
                                                        bps[k][:], op=ALU.add)
                else:
                    # -- broadcast over NI and write out --
                    nc.vector.tensor_copy(
                        orep[:].rearrange("p (n d) -> p n d", d=DC),
                        oout[:].unsqueeze(1).broadcast_to([NC, NI, DC]))
                    nc.sync.dma_start(out_d[:, :], orep[:])

    nc.compile()
    return nc


def _host_prep(x, W1, W2):
    x = np.ascontiguousarray(x, np.float32)
    W1f = np.ascontiguousarray(W1, np.float32).reshape(CP, 7, 6)
    W2f = np.ascontiguousarray(W2, np.float32).reshape(CP, 16, 16)

    blob = np.zeros((128, FBLOB), np.float32)
    for k in range(2):
        rows = slice(k * 128, (k + 1) * 128)
        blob[:, _C_W1GD + 42 * k:_C_W1GD + 42 * (k + 1)] = \
            W1f[rows].reshape(128, 42)
        blob[:, _C_W1DG + 42 * k:_C_W1DG + 42 * (k + 1)] = \
            W1f[rows].transpose(0, 2, 1).reshape(128, 42)
        blob[:, _C_W2AD + 256 * k:_C_W2AD + 256 * (k + 1)] = \
            W2f[rows].reshape(128, 256)
        blob[:, _C_W2DA + 256 * k:_C_W2DA + 256 * (k + 1)] = \
            W2f[rows].transpose(0, 2, 1).reshape(128, 256)
    cp = np.arange(CP)
    selT = (cp[:, None] // NP == np.arange(NC)[None, :]).astype(np.float32)
    for k in range(2):
        blob[:, _C_SELT + 32 * k:_C_SELT + 32 * (k + 1)] = \
            selT[k * 128:(k + 1) * 128]
    blob[:, _C_IDENT:_C_IDENT + 128] = np.eye(128, dtype=np.float32)
    blob[0:NC, _C_BSEL:_C_BSEL + 256] = selT.T

    ones = np.ones((J, 1), np.float32)
    in_maps = []
    for b in range(B):
        xb = x[b].reshape(J, ID)
        xfeat = np.concatenate([xb[:, 1:7], ones, xb[:, 7:23]], axis=1)
        bl = blob.copy()
        bl[:, 0:23] = xfeat[0:128]
        bl[:, 23:46] = xfeat[128:256]
        in_maps.append({
            "blob": bl,
            "xfT": np.ascontiguousarray(xfeat.T).astype(ml_dtypes.bfloat16),
        })
    return in_maps


def run(x, W1, W2, **run_kwargs):
    if "nc" not in _CACHE:
        _CACHE["nc"] = _build_nc()
    nc = _CACHE["nc"]
    in_maps = _host_prep(x, W1, W2)
    res = bass_utils.run_bass_kernel_spmd(nc, in_maps,
                                          core_ids=list(range(N_CORES)),
                                          **run_kwargs)
    out = np.stack([res.results[i]["out"].reshape(NC, NI, DC)
                    for i in range(N_CORES)]).astype(np.float32)
    return out, res


def kernel(x, W1, W2):
    out, _ = run(x, W1, W2)
    return out


# revision 15
# speedup vs baseline: 1.0719x; 1.0719x over previous
"""Trainium2 Bass kernel for the CAN capsule-routing module (nn_CAN_12446815223813).

Self-contained: hardcodes problem shapes, shards batch B=8 across 8 NeuronCores
(pure data parallel, one batch element per core), runs a Tile/Bass kernel via
run_bass_kernel_spmd, and gathers the full [8, 32, 8, 23] output.

Algorithm (mathematically identical to the reference, factored):
  * The NI axis of `hat` is a pure broadcast and the joint softmax over
    (NI,NC,NP) just scales the denominator by NI=8; outputs are identical
    across NI, so routing runs once per (NC,NP) and the result is broadcast.
  * ppart (x[...,0]) never reaches the output -> dropped.
  * hat[B,M,K,DC] is never materialized. With xfeat[j,f] = [x_geo(6), 1,
    x_attr(16)] (j = IC*II positions, f = 23 feats) and W1f[cp,7,6],
    W2f[cp,16,16] (cp = NC*NP):
      per routing iter (b-logits kept transposed as b[j, cp], accumulated
      directly in PSUM by the agree matmuls):
        e = exp(b);  r = 1/(8 * rowsum(e))          (|b| <= ~12: no max-sub)
        CF[cp,:] = e.T @ (xfeat * r/8)              (bf16 matmul, f32 psum)
        Sga[cp,:] = [cg @ W1f[cp], ca @ W2f[cp], sumc]   (DVE mul+reduce)
        O23[c,:]  = sum_p Sga[(c,p),:]              (ones-selector matmul)
        outputs   = squash(O23)    (rsqrt via exp(-0.5*ln(.)): one ACT table)
        V[cp,:]   = [W1f[cp] @ og(c), 0.01 * W2f[cp] @ oa(c)]
        b        += xfeat @ V.T    (bf16 matmul accumulating into PSUM)

Hardware notes (found by bisection on trn2):
  * ScalarE ACTIVATE reading PSUM with partial-partition tiles crashes the
    exec unit -> PSUM reads happen on VectorE/TensorE, except the full
    128-partition exp reads of the b PSUM tiles.
  * tensor_tensor_reduce crashes the exec unit -> mul + reduce pair instead.
  * All constant inputs arrive as one [128, F] blob -> single input DMA.
"""

import numpy as np
import ml_dtypes

import concourse.bass as bass
import concourse.bacc as bacc
import concourse.tile as tile
from concourse import bass_utils, mybir

F32 = mybir.dt.float32
BF16 = mybir.dt.bfloat16
AF = mybir.ActivationFunctionType
ALU = mybir.AluOpType
AX = mybir.AxisListType

B, IC, II, ID = 8, 16, 16, 23
NC, NP, DA = 32, 8, 16
NI = 8
DIM_GEOM = 6
DC = 1 + DIM_GEOM + DA  # 23
J = IC * II             # 256 positions
CP = NC * NP            # 256 capsule-part pairs
EPS = 1e-7
ROUTINGS = 3
N_CORES = 8

# blob column layout (f32, [128, FBLOB])
_C_XF = 0                    # xf chunks: [0:23], [23:46]
_C_W1GD = 46                 # w1 [p,g,d]: 46:88, 88:130
_C_W1DG = 130                # w1 [p,d,g]: 130:172, 172:214
_C_W2AD = 214                # w2 [p,a,d]: 214:470, 470:726
_C_W2DA = 726                # w2 [p,d,a]: 726:982, 982:1238
_C_SELT = 1238               # selT: 1238:1270, 1270:1302
_C_IDENT = 1302              # identity: 1302:1430
_C_BSEL = 1430               # bsel (rows 0:32): 1430:1686
FBLOB = 1686

_CACHE = {}

# accumulate b-logits in PSUM (HW-verified; CoreSim's accumulation-group
# model rejects mid-group PSUM reads, so simtest sets this False).
# NOTE: HW-verified that ScalarE exp reading PSUM crashes the exec unit
# (NRT_EXEC_UNIT_UNRECOVERABLE 101), so this stays False.
PSUM_B = False


def _build_nc(psum_b=None):
    psum_b = PSUM_B if psum_b is None else psum_b
    nc = bacc.Bacc("TRN2", target_bir_lowering=False, debug=False,
                   num_devices=N_CORES)

    blob_d = nc.dram_tensor("blob", [128, FBLOB], F32, kind="ExternalInput")
    xfT_d = nc.dram_tensor("xfT", [DC, J], BF16, kind="ExternalInput")
    out_d = nc.dram_tensor("out", [NC, NI * DC], F32, kind="ExternalOutput")

    with tile.TileContext(nc) as tc:
        with (
            tc.tile_pool(name="sb", bufs=1) as sb,
            tc.tile_pool(name="pmm", bufs=2, space="PSUM") as pmm,   # CF/OGA [128,23]
            tc.tile_pool(name="po", bufs=1, space="PSUM") as po,     # O23 [32,23]
            tc.tile_pool(name="pvt", bufs=2, space="PSUM") as pvt,   # VT [23,128]
            tc.tile_pool(name="pb", bufs=1, space="PSUM") as pb,     # b logits [128,256]
        ):
            # ---- persistent SBUF tiles ----
            mega = sb.tile([128, FBLOB], F32, name="mega", tag="mega")
            xfT = sb.tile([DC, J], BF16, name="xfT", tag="xfT")

            e = [sb.tile([128, CP], BF16, name=f"e{k}", tag=f"e{k}") for k in range(2)]
            xfr = [sb.tile([128, DC], BF16, name=f"xfr{k}", tag=f"xfr{k}") for k in range(2)]
            rsum = [sb.tile([128, 1], F32, name=f"rsum{k}", tag=f"rsum{k}") for k in range(2)]
            rrec = [sb.tile([128, 1], F32, name=f"rrec{k}", tag=f"rrec{k}") for k in range(2)]
            tmpg = [sb.tile([128, 42], F32, name=f"tmpg{k}", tag=f"tmpg{k}") for k in range(2)]
            tmpa = [sb.tile([128, 256], F32, name=f"tmpa{k}", tag=f"tmpa{k}") for k in range(2)]
            sga = [sb.tile([128, DC], F32, name=f"sga{k}", tag=f"sga{k}") for k in range(2)]
            vt_s = [sb.tile([128, DC], F32, name=f"vt{k}", tag=f"vt{k}") for k in range(2)]
            vT = sb.tile([DC, CP], BF16, name="vT", tag="vT")
            o23 = sb.tile([NC, DC], F32, name="o23", tag="o23")
            oout = sb.tile([NC, DC], F32, name="oout", tag="oout")
            orep = sb.tile([NC, NI * DC], F32, name="orep", tag="orep")
            sc_r = sb.tile([NC, 1], F32, name="sc_r", tag="sc_r")
            s2 = sb.tile([NC, 1], F32, name="s2", tag="s2")
            sq = sb.tile([NC, 16], F32, name="sq", tag="sq")
            t1 = sb.tile([NC, 1], F32, name="t1", tag="t1")
            r1 = sb.tile([NC, 1], F32, name="r1", tag="r1")
            lnt = sb.tile([NC, 1], F32, name="lnt", tag="lnt")
            r2 = sb.tile([NC, 1], F32, name="r2", tag="r2")
            epsb = sb.tile([NC, 1], F32, name="epsb", tag="epsb")
            I32 = mybir.dt.int32
            magic = sb.tile([NC, 1], I32, name="magic", tag="magic")
            xs2 = sb.tile([NC, 1], F32, name="xs2", tag="xs2")
            ihal = sb.tile([NC, 1], I32, name="ihal", tag="ihal")
            y0 = sb.tile([NC, 1], F32, name="y0", tag="y0")
            yt = sb.tile([NC, 1], F32, name="yt", tag="yt")
            zerob = sb.tile([128, 1], F32, name="zerob", tag="zerob")

            # b logits live in PSUM; agree matmuls accumulate into them
            bps = [pb.tile([128, CP], F32, name=f"bps{k}", tag=f"bps{k}")
                   for k in range(2)]
            if not psum_b:
                bT = [sb.tile([128, CP], F32, name=f"bT{k}", tag=f"bT{k}")
                      for k in range(2)]

            # ---- load all inputs in two DMAs ----
            nc.sync.dma_start(mega[:], blob_d[:, :])
            nc.sync.dma_start(xfT[:], xfT_d[:, :])

            # views into the blob
            def col(c0, w):
                return mega[:, c0:c0 + w]
            xf = [col(_C_XF + 23 * k, 23) for k in range(2)]
            w1_gd = [col(_C_W1GD + 42 * k, 42).rearrange("p (g d) -> p g d", d=6)
                     for k in range(2)]
            w1_dg = [col(_C_W1DG + 42 * k, 42).rearrange("p (d g) -> p d g", g=7)
                     for k in range(2)]
            w2_ad = [col(_C_W2AD + 256 * k, 256).rearrange("p (a d) -> p a d", d=16)
                     for k in range(2)]
            w2_da = [col(_C_W2DA + 256 * k, 256).rearrange("p (d a) -> p d a", a=16)
                     for k in range(2)]
            selT = [col(_C_SELT + 32 * k, 32) for k in range(2)]
            ident = col(_C_IDENT, 128)
            bsel = mega[0:NC, _C_BSEL:_C_BSEL + 256]

            nc.gpsimd.memset(epsb[:], EPS)
            nc.gpsimd.memset(magic[:], 0x5F3759DF)
            nc.gpsimd.memset(zerob[:], 0.0)
            # t=0: uniform routing weights (b=0) -> e=1, r=1/(8*256)
            for k in range(2):
                nc.gpsimd.memset(e[k][:], 1.0)

            for t in range(ROUTINGS):
                # -- normalized position weights --
                if t == 0:
                    for k in range(2):
                        nc.vector.tensor_scalar_mul(xfr[k][:], xf[k],
                                                    1.0 / (NI * CP))
                else:
                    for k in range(2):
                        # e = exp(b) straight from PSUM; rowsum via accumulator
                        nc.scalar.activation(e[k][:],
                                             bps[k][:] if psum_b else bT[k][:],
                                             AF.Exp, bias=zerob[:], scale=1.0,
                                             accum_out=rsum[k][:])
                        nc.vector.reciprocal(rrec[k][:], rsum[k][:])
                        # xfr = xf * rrec / NI
                        nc.vector.tensor_scalar(xfr[k][:], xf[k], rrec[k][:],
                                                1.0 / NI, op0=ALU.mult,
                                                op1=ALU.mult)

                # -- CF[cp, f] = sum_j c[cp,j] * xfeat[j,f]  (e.T @ xfr) --
                cf = []
                for m in range(2):
                    cfm = pmm.tile([128, DC], F32, name="pmm", tag="pmm")
                    for k in range(2):
                        nc.tensor.matmul(cfm[:], e[k][:, m * 128:(m + 1) * 128],
                                         xfr[k][:], start=(k == 0), stop=(k == 1))
                    cf.append(cfm)

                # -- Sga[cp] = [cg @ W1f, ca @ W2f, sumc] --
                for m in range(2):
                    cg_b = cf[m][:, 0:7].unsqueeze(1).broadcast_to([128, 6, 7])
                    nc.vector.scalar_tensor_tensor(
                        tmpg[m][:].rearrange("p (d g) -> p d g", g=7),
                        cg_b, 1.0, w1_dg[m], op0=ALU.mult, op1=ALU.mult)
                    nc.vector.tensor_reduce(
                        sga[m][:, 0:6], tmpg[m][:].rearrange("p (d g) -> p d g", g=7),
                        axis=AX.X, op=ALU.add)
                    ca_b = cf[m][:, 7:23].unsqueeze(1).broadcast_to([128, 16, 16])
                    nc.vector.scalar_tensor_tensor(
                        tmpa[m][:].rearrange("p (d a) -> p d a", a=16),
                        ca_b, 1.0, w2_da[m], op0=ALU.mult, op1=ALU.mult)
                    nc.vector.tensor_reduce(
                        sga[m][:, 6:22], tmpa[m][:].rearrange("p (d a) -> p d a", a=16),
                        axis=AX.X, op=ALU.add)
                    nc.vector.tensor_copy(sga[m][:, 22:23], cf[m][:, 6:7])

                # -- O23[c] = sum_p Sga[(c,p)] --
                o23p = po.tile([NC, DC], F32, name="po", tag="po")
                for m in range(2):
                    nc.tensor.matmul(o23p[:], selT[m], sga[m][:],
                                     start=(m == 0), stop=(m == 1))
                nc.vector.tensor_copy(o23[:], o23p[:])

                # -- squash -> outputs [32, 23] --
                nc.vector.reciprocal(sc_r[:], o23[:, 22:23])
                nc.vector.tensor_scalar_mul(oout[:, 1:7], o23[:, 0:6], sc_r[:])
                nc.vector.tensor_tensor(sq[:], o23[:, 6:22], o23[:, 6:22],
                                        op=ALU.mult)
                nc.vector.reduce_sum(s2[:], sq[:], axis=AX.X)
                nc.vector.tensor_scalar_add(t1[:], s2[:], 1.0)
                nc.vector.reciprocal(r1[:], t1[:])
                # r2 = rsqrt(s2+eps) via Quake bit-hack + 2 Newton steps
                # (keeps ScalarE exp-only: a single ACT table set, no reloads)
                nc.vector.tensor_scalar_add(xs2[:], s2[:], EPS)
                nc.vector.tensor_scalar(ihal[:], xs2[:].bitcast(I32), 1, None,
                                        op0=ALU.logical_shift_right)
                nc.vector.tensor_tensor(y0[:].bitcast(I32), magic[:], ihal[:],
                                        op=ALU.subtract)
                for _ in range(2):
                    nc.vector.tensor_tensor(yt[:], xs2[:], y0[:], op=ALU.mult)
                    nc.vector.tensor_tensor(yt[:], yt[:], y0[:], op=ALU.mult)
                    nc.vector.tensor_scalar(yt[:], yt[:], -0.5, 1.5,
                                            op0=ALU.mult, op1=ALU.add)
                    nc.vector.tensor_tensor(y0[:], y0[:], yt[:], op=ALU.mult)
                r2 = y0
                # scale = s2 * r1 * r2, written straight into outputs col 0
                nc.vector.scalar_tensor_tensor(oout[:, 0:1], r1[:], s2[:], r2[:],
                                               op0=ALU.mult, op1=ALU.mult)
                nc.vector.tensor_scalar_mul(oout[:, 7:23], o23[:, 6:22],
                                            oout[:, 0:1])

                if t < ROUTINGS - 1:
                    # -- V[cp] = [W1f @ og, 0.01 * W2f @ oa] --
                    for m in range(2):
                        ogam = pmm.tile([128, DC], F32, name="pmm", tag="pmm")
                        nc.tensor.matmul(ogam[:], bsel[:, m * 128:(m + 1) * 128],
                                         oout[:], start=True, stop=True)
                        og_b = ogam[:, 1:7].unsqueeze(1).broadcast_to([128, 7, 6])
                        nc.vector.scalar_tensor_tensor(
                            tmpg[m][:].rearrange("p (g d) -> p g d", d=6),
                            og_b, 1.0, w1_gd[m], op0=ALU.mult, op1=ALU.mult)
                        nc.vector.tensor_reduce(
                            vt_s[m][:, 0:7],
                            tmpg[m][:].rearrange("p (g d) -> p g d", d=6),
                            axis=AX.X, op=ALU.add)
                        oa_b = ogam[:, 7:23].unsqueeze(1).broadcast_to([128, 16, 16])
                        nc.vector.scalar_tensor_tensor(
                            tmpa[m][:].rearrange("p (a d) -> p a d", d=16),
                            oa_b, 0.01, w2_ad[m], op0=ALU.mult, op1=ALU.mult)
                        nc.vector.tensor_reduce(
                            vt_s[m][:, 7:23],
                            tmpa[m][:].rearrange("p (a d) -> p a d", d=16),
                            axis=AX.X, op=ALU.add)
                        # transpose V chunk -> vT[:, chunk] (bf16 for agree matmul)
                        vtp = pvt.tile([DC, 128], F32, name="pvt", tag="pvt")
                        nc.tensor.transpose(vtp[:], vt_s[m][:], ident)
                        nc.vector.tensor_copy(vT[:, m * 128:(m + 1) * 128], vtp[:])

                    # -- agree[j, cp] = xfeat @ V.T, accumulated into b PSUM --
                    for k in range(2):
                        if psum_b:
                            nc.tensor.matmul(bps[k][:],
                                             xfT[:, k * 128:(k + 1) * 128],
                                             vT[:], start=(t == 0),
                                             stop=(t == 1),
                                             skip_group_check=True)
                        else:
                            nc.tensor.matmul(bps[k][:],
                                             xfT[:, k * 128:(k + 1) * 128],
                                             vT[:], start=True, stop=True)
                            if t == 0:
                                nc.vector.tensor_copy(bT[k][:], bps[k][:])
                            else:
                                nc.vector.tensor_tensor(bT[k][:], bT[k][:],
                                                        bps[k][:], op=ALU.add)
                else:
                    # -- broadcast over NI and write out --
                    nc.vector.tensor_copy(
                        orep[:].rearrange("p (n d) -> p n d", d=DC),
                        oout[:].unsqueeze(1).broadcast_to([NC, NI, DC]))
                    nc.sync.dma_start(out_d[:, :], orep[:])

    nc.compile()
    return nc


def _host_prep(x, W1, W2):
    x = np.ascontiguousarray(x, np.float32)
    W1f = np.ascontiguousarray(W1, np.float32).reshape(CP, 7, 6)
    W2f = np.ascontiguousarray(W2, np.float32).reshape(CP, 16, 16)

    blob = np.zeros((128, FBLOB), np.float32)
    for k in range(2):
        rows = slice(k * 128, (k + 1) * 128)
        blob[:, _C_W1GD + 42 * k:_C_W1GD + 42 * (k + 1)] = \
            W1f[rows].reshape(128, 42)
        blob[:, _C_W1DG + 42 * k:_C_W1DG + 42 * (k + 1)] = \
            W1f[rows].transpose(0, 2, 1).reshape(128, 42)
        blob[:, _C_W2AD + 256 * k:_C_W2AD + 256 * (k + 1)] = \
            W2f[rows].reshape(128, 256)
        blob[:, _C_W2DA + 256 * k:_C_W2DA + 256 * (k + 1)] = \
            W2f[rows].transpose(0, 2, 1).reshape(128, 256)
    cp = np.arange(CP)
    selT = (cp[:, None] // NP == np.arange(NC)[None, :]).astype(np.float32)
    for k in range(2):
        blob[:, _C_SELT + 32 * k:_C_SELT + 32 * (k + 1)] = \
            selT[k * 128:(k + 1) * 128]
    blob[:, _C_IDENT:_C_IDENT + 128] = np.eye(128, dtype=np.float32)
    blob[0:NC, _C_BSEL:_C_BSEL + 256] = selT.T

    ones = np.ones((J, 1), np.float32)
    in_maps = []
    for b in range(B):
        xb = x[b].reshape(J, ID)
        xfeat = np.concatenate([xb[:, 1:7], ones, xb[:, 7:23]], axis=1)
        bl = blob.copy()
        bl[:, 0:23] = xfeat[0:128]
        bl[:, 23:46] = xfeat[128:256]
        in_maps.append({
            "blob": bl,
            "xfT": np.ascontiguousarray(xfeat.T).astype(ml_dtypes.bfloat16),
        })
    return in_maps


def run(x, W1, W2, **run_kwargs):
    if "nc" not in _CACHE:
        _CACHE["nc"] = _build_nc()
    nc = _CACHE["nc"]
    in_maps = _host_prep(x, W1, W2)
    res = bass_utils.run_bass_kernel_spmd(nc, in_maps,
                                          core_ids=list(range(N_CORES)),
                                          **run_kwargs)
    out = np.stack([res.results[i]["out"].reshape(NC, NI, DC)
                    for i in range(N_CORES)]).astype(np.float32)
    return out, res


def kernel(x, W1, W2):
    out, _ = run(x, W1, W2)
    return out
